# revision 21
# baseline (speedup 1.0000x reference)
"""Trainium2 Bass kernel for DiverseSiblingsSearch (per-beam top-k + sibling
penalty + cross-beam top-k).

Contract: kernel(**inputs) takes the FULL inputs (lprobs [128,5,50257] f32,
scores [128,5,10] f32, step scalar) and returns the FULL outputs
(final_scores [128,10] f32, final_indices [128,10] i32, final_beams [128,10] i32).

Sharding: pure data parallel over the batch dim — 16 batches (80 beam-rows)
per NeuronCore, 8 cores.

Device algorithm (per core, 80 rows x 51200 padded vocab; the full
25.7M-element scan and the top-k selection):
  A1  group-max: reduce_max over groups of 50 -> 1024 group maxes per row,
      computed in a [128 partitions, rows, 400] layout so the DVE scan uses
      all 128 partitions; DMA tiles of 16 rows multi-buffered so the scan
      hides under the HBM stream.
  A2  PE-transpose the [128, 80, 8] group-max tensor into D [80 rows, 1024]
      (group q = p*8 + g covers vocab [50q, 50q+50)), then reduce runs of 4
      into super-group maxes sgm [80, 256] (super-group covers 200 vocab).
  A3  top-16 super-groups per row via max8 / max_index / match_replace /
      max8 / max_index -> gsel [80, 16].
Host: gather the 16 winning 200-wide vocab spans per row from lprobs
(guaranteed to contain the row's top-10: any group holding a top-10 element
has group-max >= the 10th value, so winner groups are a prefix of groups
sorted by max — at most 10 of them), add the running score, exact top-10 per
row, rank penalty, cross-beam top-10 over 50, final gather. O(bsz*beam*2k)
numpy work.
"""

from contextlib import ExitStack

import ml_dtypes
import numpy as np

import concourse.bacc as bacc
import concourse.mybir as mybir
import concourse.tile as tile
from concourse.bass_utils import run_bass_kernel_spmd

# ---- geometry (hardcoded for this problem) ----
BSZ = 128
BEAM = 5
VOCAB = 50257
K = 10  # min(2*beam, beam*vocab-1)
DIVERSITY_RATE = 0.5

N_CORES = 8
B_PER_CORE = BSZ // N_CORES  # 16
R = B_PER_CORE * BEAM  # 80 rows per core
P = 128  # SBUF partitions
FPP = 400  # vocab elems per partition (padded)
VPAD = P * FPP  # 51200
GS = 50  # group size
GPP = FPP // GS  # 8 groups per partition-chunk
NG = P * GPP  # 1024 groups per row
SGF = 4  # groups per super-group
NSG = NG // SGF  # 256 super-groups per row
SGS = GS * SGF  # 200 vocab per super-group
NSEL = 16  # super-groups selected per row
RT = 16  # rows per DMA tile
NT = R // RT  # 5 tiles
NEG = -1.0e30

F32 = mybir.dt.float32
BF16 = mybir.dt.bfloat16
U32 = mybir.dt.uint32

_TRACE = False  # test.py flips this to profile
_LAST_RESULTS = None  # BassKernelResults of the last run (for test.py)


def build_nc():
    nc = bacc.Bacc(
        "TRN2", target_bir_lowering=False, debug=False, num_devices=N_CORES
    )
    lp = nc.dram_tensor("lp", [NT, P, RT * FPP], BF16, kind="ExternalInput")
    id_in = nc.dram_tensor("ident", [P, P], F32, kind="ExternalInput")
    o_gsel = nc.dram_tensor("gsel", [R, NSEL], U32, kind="ExternalOutput")

    def emit(tc, ctx):
        xpool = ctx.enter_context(tc.tile_pool(name="x", bufs=2))
        tpool = ctx.enter_context(tc.tile_pool(name="t", bufs=3))
        spool = ctx.enter_context(tc.tile_pool(name="s", bufs=1))
        ppool = ctx.enter_context(tc.tile_pool(name="p", bufs=4, space="PSUM"))

        ident = spool.tile([P, P], F32)
        nc.sync.dma_start(ident[:], id_in.ap())

        SPP = FPP // SGS  # super-groups per partition-chunk (2)
        gm = spool.tile([P, R, SPP], F32)  # super-group maxes, [p, r, g]
        # A1: stream row-tiles (bf16, host-packed into 8 blocks per
        # super-group so three tree rounds compare flat contiguous halves at
        # the DVE's 2x bf16 mode), then a 1x reduce_max over the last 25.
        TE = RT * FPP  # elems per partition per tile
        for t in range(NT):
            x = xpool.tile([P, TE], BF16, tag="x")
            nc.sync.dma_start(x[:], lp.ap()[t])
            y = tpool.tile([P, TE // 2], BF16, tag="y")
            nc.vector.tensor_tensor(
                out=y[:], in0=x[:, 0 : TE // 2], in1=x[:, TE // 2 : TE],
                op=mybir.AluOpType.max,
            )
            z = tpool.tile([P, TE // 4], BF16, tag="z")
            nc.vector.tensor_tensor(
                out=z[:], in0=y[:, 0 : TE // 4], in1=y[:, TE // 4 : TE // 2],
                op=mybir.AluOpType.max,
            )
            w = tpool.tile([P, TE // 8], BF16, tag="w")
            nc.vector.tensor_tensor(
                out=w[:], in0=z[:, 0 : TE // 8], in1=z[:, TE // 8 : TE // 4],
                op=mybir.AluOpType.max,
            )
            nc.vector.reduce_max(
                gm[:, t * RT : (t + 1) * RT, :],
                w[:].rearrange("p (r g j) -> p r g j", r=RT, g=SPP),
                axis=mybir.AxisListType.X,
            )

        # A2: transpose [p, r, g] -> sgm[r, s] with s = p*SPP + g
        # (super-group s covers vocab [200s, 200s+200)).
        sgm = spool.tile([R, NSG], F32)
        dv = sgm[:].rearrange("r (p g) -> r p g", g=SPP)
        for g in range(SPP):
            pt = ppool.tile([R, P], F32, name=f"pt{g}", tag="pt")
            nc.tensor.transpose(pt[:], gm[:, :, g], ident[:])
            nc.scalar.copy(dv[:, :, g], pt[:])

        # A3: top-16 super-groups per row
        gsel = spool.tile([R, NSEL], U32)
        mA = spool.tile([R, 8], F32)
        nc.vector.max(out=mA[:], in_=sgm[:])
        nc.vector.max_index(out=gsel[:, 0:8], in_max=mA[:], in_values=sgm[:])
        sg2 = spool.tile([R, NSG], F32)
        nc.vector.match_replace(
            out=sg2[:], in_to_replace=mA[:], in_values=sgm[:], imm_value=NEG
        )
        mB = spool.tile([R, 8], F32)
        nc.vector.max(out=mB[:], in_=sg2[:])
        nc.vector.max_index(out=gsel[:, 8:16], in_max=mB[:], in_values=sg2[:])

        nc.sync.dma_start(o_gsel.ap(), gsel[:])

    with tile.TileContext(nc) as tc, ExitStack() as ctx:
        emit(tc, ctx)

    nc.compile()
    return nc


_NC = None


def _get_nc():
    global _NC
    if _NC is None:
        _NC = build_nc()
    return _NC


def make_in_maps(lprobs):
    """Pad + shard lprobs into per-core input maps."""
    pad = np.full((BSZ, BEAM, VPAD - VOCAB), NEG, dtype=np.float32)
    lp_pad = np.concatenate([lprobs, pad], axis=-1)  # [128, 5, 51200]
    in_maps = []
    for c in range(N_CORES):
        b0, b1 = c * B_PER_CORE, (c + 1) * B_PER_CORE
        # [NT, RT, P, SPP, 4, 50] -> [NT, P, quarter, RT, SPP, 50]: both
        # tree rounds then pair elements of the same super-group while
        # reading flat contiguous halves (DVE 2x bf16 mode).
        shard = lp_pad[b0:b1].reshape(NT, RT, P, FPP // SGS, 8, SGS // 8)
        planar = np.ascontiguousarray(
            shard.transpose(0, 2, 4, 1, 3, 5).astype(ml_dtypes.bfloat16)
        ).reshape(NT, P, RT * FPP)
        in_maps.append({"lp": planar, "ident": np.eye(P, dtype=np.float32)})
    return in_maps


def postprocess(results, lprobs, scores, step):
    """Device super-group selection -> exact full outputs on host.

    The device guarantees each row's top-10 lives inside its 16 selected
    128-wide vocab spans; everything past this point is O(bsz*beam*2k).
    """
    nrows = BSZ * BEAM
    gsel = np.concatenate([r["gsel"] for r in results], axis=0).astype(
        np.int64
    )  # [640, 16] super-group ids; vocab span = [200*sg, 200*sg+200)

    lpr = lprobs.reshape(nrows, VOCAB)
    c = scores.reshape(nrows, -1)[:, step - 1].astype(np.float32)

    # gather candidate spans (clip into the real vocab; padding never wins)
    span = gsel[:, :, None] * SGS + np.arange(SGS)[None, None, :]
    span_c = np.minimum(span, VOCAB - 1).reshape(nrows, -1)
    oob = (span >= VOCAB).reshape(nrows, -1)
    cand = np.take_along_axis(lpr, span_c, axis=1)
    cand = np.where(oob, np.float32(NEG), cand)
    cand = cand + c[:, None]  # running-score offset, f32 like the reference

    # exact per-row top-10 (value desc, ties -> lower vocab id, like lax.top_k)
    vocab_ids = np.where(oob, VOCAB, span.reshape(nrows, -1))
    order = np.lexsort((vocab_ids, -cand), axis=1)[:, :K]
    top_vals = np.take_along_axis(cand, order, axis=1)  # [640, 10]
    top_vocab = np.take_along_axis(vocab_ids, order, axis=1)

    s = top_vals.reshape(BSZ, BEAM, K) - (
        np.arange(1, K + 1, dtype=np.float32) * np.float32(DIVERSITY_RATE)
    )
    s50 = s.reshape(BSZ, BEAM * K)
    indices = top_vocab.reshape(BSZ, BEAM * K)

    flat_pos = np.argsort(-s50, axis=1, kind="stable")[:, :K]
    final_scores = np.take_along_axis(s50, flat_pos, axis=1)
    final_indices = np.take_along_axis(indices, flat_pos, axis=1).astype(
        np.int32
    )
    final_beams = (flat_pos // K).astype(np.int32)
    return final_scores, final_indices, final_beams


def kernel(lprobs, scores, step):
    global _LAST_RESULTS
    lprobs = np.asarray(lprobs, dtype=np.float32)
    scores = np.asarray(scores, dtype=np.float32)
    step = int(step)
    nc = _get_nc()
    in_maps = make_in_maps(lprobs)
    res = run_bass_kernel_spmd(
        nc, in_maps, core_ids=list(range(N_CORES)), trace=_TRACE
    )
    _LAST_RESULTS = res
    return postprocess(res.results, lprobs, scores, step)


# revision 22
# speedup vs baseline: 1.0561x; 1.0561x over previous
"""Trainium2 Bass kernel for DiverseSiblingsSearch (per-beam top-k + sibling
penalty + cross-beam top-k).

Contract: kernel(**inputs) takes the FULL inputs (lprobs [128,5,50257] f32,
scores [128,5,10] f32, step scalar) and returns the FULL outputs
(final_scores [128,10] f32, final_indices [128,10] i32, final_beams [128,10] i32).

Sharding: pure data parallel over the batch dim — 16 batches (80 beam-rows)
per NeuronCore, 8 cores.

Device algorithm (per core, 80 rows x 51200 padded vocab; the full
25.7M-element scan and the top-k selection):
  A1  group-max: reduce_max over groups of 50 -> 1024 group maxes per row,
      computed in a [128 partitions, rows, 400] layout so the DVE scan uses
      all 128 partitions; DMA tiles of 16 rows multi-buffered so the scan
      hides under the HBM stream.
  A2  PE-transpose the [128, 80, 8] group-max tensor into D [80 rows, 1024]
      (group q = p*8 + g covers vocab [50q, 50q+50)), then reduce runs of 4
      into super-group maxes sgm [80, 256] (super-group covers 200 vocab).
  A3  top-16 super-groups per row via max8 / max_index / match_replace /
      max8 / max_index -> gsel [80, 16].
Host: gather the 16 winning 200-wide vocab spans per row from lprobs
(guaranteed to contain the row's top-10: any group holding a top-10 element
has group-max >= the 10th value, so winner groups are a prefix of groups
sorted by max — at most 10 of them), add the running score, exact top-10 per
row, rank penalty, cross-beam top-10 over 50, final gather. O(bsz*beam*2k)
numpy work.
"""

from contextlib import ExitStack

import ml_dtypes
import numpy as np

import concourse.bacc as bacc
import concourse.mybir as mybir
import concourse.tile as tile
from concourse.bass_utils import run_bass_kernel_spmd

# ---- geometry (hardcoded for this problem) ----
BSZ = 128
BEAM = 5
VOCAB = 50257
K = 10  # min(2*beam, beam*vocab-1)
DIVERSITY_RATE = 0.5

N_CORES = 8
B_PER_CORE = BSZ // N_CORES  # 16
R = B_PER_CORE * BEAM  # 80 rows per core
P = 128  # SBUF partitions
FPP = 400  # vocab elems per partition (padded)
VPAD = P * FPP  # 51200
GS = 50  # group size
GPP = FPP // GS  # 8 groups per partition-chunk
NG = P * GPP  # 1024 groups per row
SGF = 4  # groups per super-group
NSG = NG // SGF  # 256 super-groups per row
SGS = GS * SGF  # 200 vocab per super-group
NSEL = 16  # super-groups selected per row
TILES = [4, 8, 12, 16, 20, 20]  # rows per DMA tile (sums to R); small
# first tiles start the DVE early, big later tiles amortize overheads
assert sum(TILES) == R
NEG = -1.0e30

F32 = mybir.dt.float32
BF16 = mybir.dt.bfloat16
U32 = mybir.dt.uint32

_TRACE = False  # test.py flips this to profile
_LAST_RESULTS = None  # BassKernelResults of the last run (for test.py)


def build_nc():
    nc = bacc.Bacc(
        "TRN2", target_bir_lowering=False, debug=False, num_devices=N_CORES
    )
    lp = nc.dram_tensor("lp", [P, R * FPP], BF16, kind="ExternalInput")
    id_in = nc.dram_tensor("ident", [P, P], F32, kind="ExternalInput")
    o_gsel = nc.dram_tensor("gsel", [R, NSEL], U32, kind="ExternalOutput")

    def emit(tc, ctx):
        xpool = ctx.enter_context(tc.tile_pool(name="x", bufs=1))
        tpool = ctx.enter_context(tc.tile_pool(name="t", bufs=1))
        spool = ctx.enter_context(tc.tile_pool(name="s", bufs=1))
        ppool = ctx.enter_context(tc.tile_pool(name="p", bufs=4, space="PSUM"))

        ident = spool.tile([P, P], F32)
        nc.sync.dma_start(ident[:], id_in.ap())

        SPP = FPP // SGS  # super-groups per partition-chunk (2)
        gm = spool.tile([P, R, SPP], F32)  # super-group maxes, [p, r, g]
        # A1: stream row-tiles (bf16, host-packed into 8 blocks per
        # super-group so three tree rounds compare flat contiguous halves at
        # the DVE's 2x bf16 mode), then a 1x reduce_max over the last 25.
        r0 = 0
        for t, rt in enumerate(TILES):
            te = rt * FPP
            off = r0 * FPP
            x = xpool.tile([P, te], BF16, name=f"x{t}", tag="x", bufs=3)
            nc.sync.dma_start(x[:], lp.ap()[:, off : off + te])
            y = tpool.tile([P, te // 2], BF16, name=f"y{t}", tag="y", bufs=2)
            nc.vector.tensor_tensor(
                out=y[:], in0=x[:, 0 : te // 2], in1=x[:, te // 2 : te],
                op=mybir.AluOpType.max,
            )
            z = tpool.tile([P, te // 4], BF16, name=f"z{t}", tag="z", bufs=2)
            nc.vector.tensor_tensor(
                out=z[:], in0=y[:, 0 : te // 4], in1=y[:, te // 4 : te // 2],
                op=mybir.AluOpType.max,
            )
            w = tpool.tile([P, te // 8], BF16, name=f"w{t}", tag="w", bufs=2)
            nc.vector.tensor_tensor(
                out=w[:], in0=z[:, 0 : te // 8], in1=z[:, te // 8 : te // 4],
                op=mybir.AluOpType.max,
            )
            nc.vector.reduce_max(
                gm[:, r0 : r0 + rt, :],
                w[:].rearrange("p (r g j) -> p r g j", r=rt, g=SPP),
                axis=mybir.AxisListType.X,
            )
            r0 += rt

        # A2: transpose [p, r, g] -> sgm[r, s] with s = p*SPP + g
        # (super-group s covers vocab [200s, 200s+200)).
        sgm = spool.tile([R, NSG], F32)
        dv = sgm[:].rearrange("r (p g) -> r p g", g=SPP)
        for g in range(SPP):
            pt = ppool.tile([R, P], F32, name=f"pt{g}", tag="pt")
            nc.tensor.transpose(pt[:], gm[:, :, g], ident[:])
            nc.scalar.copy(dv[:, :, g], pt[:])

        # A3: top-16 super-groups per row
        gsel = spool.tile([R, NSEL], U32)
        mA = spool.tile([R, 8], F32)
        nc.vector.max(out=mA[:], in_=sgm[:])
        nc.vector.max_index(out=gsel[:, 0:8], in_max=mA[:], in_values=sgm[:])
        sg2 = spool.tile([R, NSG], F32)
        nc.vector.match_replace(
            out=sg2[:], in_to_replace=mA[:], in_values=sgm[:], imm_value=NEG
        )
        mB = spool.tile([R, 8], F32)
        nc.vector.max(out=mB[:], in_=sg2[:])
        nc.vector.max_index(out=gsel[:, 8:16], in_max=mB[:], in_values=sg2[:])

        nc.sync.dma_start(o_gsel.ap(), gsel[:])

    with tile.TileContext(nc) as tc, ExitStack() as ctx:
        emit(tc, ctx)

    nc.compile()
    return nc


_NC = None


def _get_nc():
    global _NC
    if _NC is None:
        _NC = build_nc()
    return _NC


def make_in_maps(lprobs):
    """Pad + shard lprobs into per-core input maps."""
    pad = np.full((BSZ, BEAM, VPAD - VOCAB), NEG, dtype=np.float32)
    lp_pad = np.concatenate([lprobs, pad], axis=-1)  # [128, 5, 51200]
    in_maps = []
    for c in range(N_CORES):
        b0, b1 = c * B_PER_CORE, (c + 1) * B_PER_CORE
        # per tile: [rt, P, SPP, 8, 25] -> [P, block, rt, SPP, 25]: the
        # three tree rounds pair elements of the same super-group while
        # reading flat contiguous halves (DVE 2x bf16 mode).
        shard = lp_pad[b0:b1].reshape(R, P, FPP // SGS, 8, SGS // 8)
        parts, r0 = [], 0
        for rt in TILES:
            blk = shard[r0 : r0 + rt].transpose(1, 3, 0, 2, 4)
            parts.append(blk.reshape(P, rt * FPP))
            r0 += rt
        planar = np.ascontiguousarray(
            np.concatenate(parts, axis=1).astype(ml_dtypes.bfloat16)
        )
        in_maps.append({"lp": planar, "ident": np.eye(P, dtype=np.float32)})
    return in_maps


def postprocess(results, lprobs, scores, step):
    """Device super-group selection -> exact full outputs on host.

    The device guarantees each row's top-10 lives inside its 16 selected
    128-wide vocab spans; everything past this point is O(bsz*beam*2k).
    """
    nrows = BSZ * BEAM
    gsel = np.concatenate([r["gsel"] for r in results], axis=0).astype(
        np.int64
    )  # [640, 16] super-group ids; vocab span = [200*sg, 200*sg+200)

    lpr = lprobs.reshape(nrows, VOCAB)
    c = scores.reshape(nrows, -1)[:, step - 1].astype(np.float32)

    # gather candidate spans (clip into the real vocab; padding never wins)
    span = gsel[:, :, None] * SGS + np.arange(SGS)[None, None, :]
    span_c = np.minimum(span, VOCAB - 1).reshape(nrows, -1)
    oob = (span >= VOCAB).reshape(nrows, -1)
    cand = np.take_along_axis(lpr, span_c, axis=1)
    cand = np.where(oob, np.float32(NEG), cand)
    cand = cand + c[:, None]  # running-score offset, f32 like the reference

    # exact per-row top-10 (value desc, ties -> lower vocab id, like lax.top_k)
    vocab_ids = np.where(oob, VOCAB, span.reshape(nrows, -1))
    order = np.lexsort((vocab_ids, -cand), axis=1)[:, :K]
    top_vals = np.take_along_axis(cand, order, axis=1)  # [640, 10]
    top_vocab = np.take_along_axis(vocab_ids, order, axis=1)

    s = top_vals.reshape(BSZ, BEAM, K) - (
        np.arange(1, K + 1, dtype=np.float32) * np.float32(DIVERSITY_RATE)
    )
    s50 = s.reshape(BSZ, BEAM * K)
    indices = top_vocab.reshape(BSZ, BEAM * K)

    flat_pos = np.argsort(-s50, axis=1, kind="stable")[:, :K]
    final_scores = np.take_along_axis(s50, flat_pos, axis=1)
    final_indices = np.take_along_axis(indices, flat_pos, axis=1).astype(
        np.int32
    )
    final_beams = (flat_pos // K).astype(np.int32)
    return final_scores, final_indices, final_beams


def kernel(lprobs, scores, step):
    global _LAST_RESULTS
    lprobs = np.asarray(lprobs, dtype=np.float32)
    scores = np.asarray(scores, dtype=np.float32)
    step = int(step)
    nc = _get_nc()
    in_maps = make_in_maps(lprobs)
    res = run_bass_kernel_spmd(
        nc, in_maps, core_ids=list(range(N_CORES)), trace=_TRACE
    )
    _LAST_RESULTS = res
    return postprocess(res.results, lprobs, scores, step)
